# revision 10
# baseline (speedup 1.0000x reference)
"""Trainium2 Bass kernel for nn_BackwardCompatibleLoss.

Strategy (data-parallel over batch rows, 8 NeuronCores):

Host side (data movement only):
  - Rows are sorted by target label (the loss is permutation-invariant over
    batch rows).  After sorting, every same-label group is a contiguous row
    range, so for each core's 512-row shard all same-label partners lie in a
    fixed-size "window" of rows around the shard.
  - Each core receives its window of feat/feat_old rows (bf16), the window/
    local targets (f32), a per-core 0/1 weight vector (0 on its window rows)
    and an identity matrix constant.

Device side (all O(B*D) and O(B^2) math):
  - Each core L2-normalizes its window rows (bn_stats -> bn_aggr ->
    64/||x|| = exp(-0.5*ln(ss) + ln 64)).  The exp/ln rsqrt keeps ScalarE on
    a single activation table for the whole kernel (exp/ln/copy share one
    table; sqrt does not, and every table switch costs 1.3us).
    (tensor_tensor_reduce would fuse the sum of squares, but that
    instruction reproducibly hangs the device in this program, so bn_stats
    it is.)
  - Normalized rows are written bf16, DMA-transposed to [D, rows] layout,
    then cast to fp8 e4m3 (values <= 16, inside e4m3 range).
  - The local 512-row slice of feat_old is processed FIRST and AllGathered
    immediately; feat follows in a second AllGather.  The n2o sweep only
    needs the first gather, so it overlaps the second gather; the window
    pass (local data) overlaps the first.
  - Main compute per pair of 128-row j-tiles (transposed orientation
    S^T[j, i]): PSUM q = 4096*S via fp8 DoubleRow matmuls (2 contraction
    slabs per matmul); one exp over the [128, 1024] pair on ScalarE
    (E = exp(q/40.96 - 35); the -35 shift keeps every exponent in range);
    Z[1, 512] accumulates in PSUM via weights-vector matmuls
    (partition-axis reduction on the TensorEngine).
  - Window j-tiles take the same-label additive mask (-1e9), built on-device
    from target equality, before the exp; global-sweep tiles are weighted by
    w (0 on window rows) so each j contributes exactly once.  The window
    pass E tiles are computed early (they only need local data) but their
    Z-matmuls are deferred to the end of the PSUM accumulation chain, so the
    global sweep never waits on them.
  - The positive logit is the diagonal of the window n2o product (identity
    mask + ones-matmul).  loss_i = ln(Z_i) + 35 - q_pos_i/40.96, summed to a
    per-core partial.

  Top-k(1024) in the reference is replaced by the full masked logsumexp: with
  temperature 0.01 the excluded tail contributes ~2e-6 relative error.  The
  fp8 feature quantization adds ~1e-3 relative error on the loss.

Host sums the 8 partial outputs -> mean.
"""

import sys

if "/opt/trn_rl_repo" not in sys.path:
    sys.path.insert(0, "/opt/trn_rl_repo")

import math
from contextlib import ExitStack

import numpy as np

import concourse.bacc as bacc
import concourse.bass as bass
import concourse.tile as tile
from concourse import mybir
from concourse.bass_utils import run_bass_kernel_spmd

F32 = mybir.dt.float32
BF16 = mybir.dt.bfloat16
FP8 = mybir.dt.float8e4
NP_BF16 = mybir.dt.np(BF16)
AF = mybir.ActivationFunctionType
ALU = mybir.AluOpType
DR = mybir.MatmulPerfMode.DoubleRow

B, D = 4096, 512
NCORES = 8
BL = B // NCORES          # 512 local rows per core
NDB = D // 128            # 4 contraction blocks
NGT = B // 128            # 32 global j-tiles
TEMP = 0.01
QS = 64.0                 # fp8 feature scale: q = 4096 * S
SCALE_Q = (1.0 / TEMP) / (QS * QS)   # exp scale on raw psum
EBIAS = -35.0             # exp(q*SCALE_Q - 35): keeps exponents in f32 range
NEG = -1.0e9

_cache = {}


def _build(wtiles: int):
    """Build + compile the SPMD program. wtiles = window size in 128-row tiles."""
    WIN = wtiles * 128
    LPAD = ((wtiles - 4) // 2) * 128          # rows of left padding in window
    LT = LPAD // 128

    nc = bacc.Bacc("TRN2", target_bir_lowering=False, debug=False,
                   num_devices=NCORES)

    xw = nc.dram_tensor("xw", [WIN, D], BF16, kind="ExternalInput")
    yw = nc.dram_tensor("yw", [WIN, D], BF16, kind="ExternalInput")
    tw = nc.dram_tensor("tw", [WIN], F32, kind="ExternalInput")
    tl = nc.dram_tensor("tl", [BL], F32, kind="ExternalInput")
    wv = nc.dram_tensor("wv", [B], BF16, kind="ExternalInput")
    idm = nc.dram_tensor("idm", [128, 128], F32, kind="ExternalInput")
    outp = nc.dram_tensor("outp", [1, 1], F32, kind="ExternalOutput")

    natf = nc.dram_tensor("natf", [WIN, D], BF16)
    nato = nc.dram_tensor("nato", [WIN, D], BF16)
    ccin_o = nc.dram_tensor("ccin_o", [D, BL], FP8)
    ccin_f = nc.dram_tensor("ccin_f", [D, BL], FP8)
    ccout_o = nc.dram_tensor("ccout_o", [NCORES, D, BL], FP8,
                             addr_space="Shared")
    ccout_f = nc.dram_tensor("ccout_f", [NCORES, D, BL], FP8,
                             addr_space="Shared")

    with ExitStack() as ctx:
        tc = ctx.enter_context(tile.TileContext(nc))
        singles = ctx.enter_context(tc.tile_pool(name="singles", bufs=1))
        work = ctx.enter_context(tc.tile_pool(name="work", bufs=3))
        epool = ctx.enter_context(tc.tile_pool(name="epool", bufs=3))
        wE = ctx.enter_context(tc.tile_pool(name="wE", bufs=6))
        psS = ctx.enter_context(tc.tile_pool(name="psS", bufs=3, space="PSUM"))
        psA = ctx.enter_context(tc.tile_pool(name="psA", bufs=1, space="PSUM"))

        # persistent SBUF tensors
        foT = singles.tile([128, NDB, WIN], BF16, tag="foT")
        fnT = singles.tile([128, NDB, WIN], BF16, tag="fnT")
        foTq = singles.tile([128, NDB, WIN], FP8, tag="foTq")
        fnTq = singles.tile([128, NDB, WIN], FP8, tag="fnTq")
        gTo = singles.tile([128, NDB, B], FP8, tag="gTo")
        gTn = singles.tile([128, NDB, B], FP8, tag="gTn")
        tlb = singles.tile([128, BL], F32, tag="tlb")
        twc = singles.tile([128, wtiles], F32, tag="twc")
        wcol = singles.tile([128, NGT], BF16, tag="wcol")
        identS = singles.tile([128, 128], F32, tag="identS")
        ones_bf = singles.tile([128, 1], BF16, tag="ones_bf")
        ones_f = singles.tile([128, 1], F32, tag="ones_f")
        ebias = singles.tile([128, 1], F32, tag="ebias")
        ln64 = singles.tile([128, 1], F32, tag="ln64")
        psZ = psA.tile([1, BL], F32, tag="psZ")
        psP = psA.tile([1, BL], F32, tag="psP")

        nc.vector.memset(ones_bf, 1.0)
        nc.vector.memset(ebias, EBIAS)
        nc.vector.memset(ones_f, 1.0)
        nc.vector.memset(ln64, float(math.log(QS)))
        nc.sync.dma_start(out=identS, in_=idm[:, :])
        tl_ap = tl.ap()
        nc.sync.dma_start(
            out=tlb,
            in_=bass.AP(tensor=tl_ap.tensor, offset=tl_ap.offset,
                        ap=[[0, 128]] + list(tl_ap.ap)),
        )
        nc.sync.dma_start(out=twc, in_=tw.ap().rearrange("(s p) -> p s", p=128))
        nc.sync.dma_start(out=wcol, in_=wv.ap().rearrange("(g p) -> p g", p=128))

        def norm_block(src, nat, b):
            """L2-normalize rows [b*128, (b+1)*128) of src, scaled by QS."""
            xb = work.tile([128, D], BF16, tag="xb")
            nc.sync.dma_start(out=xb, in_=src[b * 128:(b + 1) * 128, :])
            st = work.tile([128, 6], F32, tag="st")
            nc.vector.bn_stats(out=st, in_=xb)
            mv = work.tile([128, 2], F32, tag="mv")
            nc.vector.bn_aggr(out=mv, in_=st)
            m2 = work.tile([128, 1], F32, tag="m2")
            nc.vector.tensor_mul(out=m2, in0=mv[:, 0:1], in1=mv[:, 0:1])
            ex2 = work.tile([128, 1], F32, tag="ex2")
            nc.vector.tensor_add(out=ex2, in0=m2, in1=mv[:, 1:2])
            lss = work.tile([128, 1], F32, tag="lss")
            nc.scalar.activation(out=lss, in_=ex2, func=AF.Ln,
                                 scale=float(D))
            rs = work.tile([128, 1], F32, tag="rs")
            # rs = exp(-0.5*ln(D*ex2) + ln64) = QS / ||x||
            nc.scalar.activation(out=rs, in_=lss, func=AF.Exp,
                                 bias=ln64, scale=-0.5)
            nb = work.tile([128, D], BF16, tag="nb")
            nc.vector.tensor_scalar_mul(out=nb, in0=xb, scalar1=rs)
            nc.sync.dma_start(out=nat[b * 128:(b + 1) * 128, :], in_=nb)

        def transpose_rows(nat, dstT, r0, r1):
            for db in range(NDB):
                nc.sync.dma_start_transpose(
                    out=dstT[:, db, r0:r1],
                    in_=nat[r0:r1, db * 128:(db + 1) * 128])

        def cast_q(srcT, dstTq, r0, r1):
            nc.scalar.activation(out=dstTq[:, :, r0:r1],
                                 in_=srcT[:, :, r0:r1], func=AF.Copy)

        # ---- Phase A+B: local rows first (fo then fn), AllGather each as
        # soon as its ccin is written; pads afterwards ----
        for src, nat, natT, natTq, ccin, ccout in (
                (yw, nato, foT, foTq, ccin_o, ccout_o),
                (xw, natf, fnT, fnTq, ccin_f, ccout_f)):
            for s in range(LT, LT + 4):
                norm_block(src, nat, s)
            transpose_rows(nat, natT, LPAD, LPAD + BL)
            cast_q(natT, natTq, LPAD, LPAD + BL)
            nc.sync.dma_start(
                out=ccin.ap().rearrange("(a p) j -> p a j", p=128),
                in_=natTq[:, :, LPAD:LPAD + BL])
            nc.gpsimd.collective_compute(
                "AllGather",
                ALU.bypass,
                replica_groups=[list(range(NCORES))],
                ins=[ccin.ap().opt()],
                outs=[ccout.ap().opt()],
            )
        pads = [s for s in range(wtiles) if not (LT <= s < LT + 4)]
        for src, nat, natT, natTq in ((yw, nato, foT, foTq),
                                      (xw, natf, fnT, fnTq)):
            for s in pads:
                norm_block(src, nat, s)
            for s in pads:
                transpose_rows(nat, natT, s * 128, (s + 1) * 128)
                cast_q(natT, natTq, s * 128, (s + 1) * 128)

        rhs_loc = fnTq[:, :, LPAD:LPAD + BL]   # [128, NDB, 512] local fn cols

        def mm_group(ps, lhs_src, j0):
            """ps[j 128, i 512] = sum_d lhs_src[d, j0:j0+128] * local fn."""
            for p in range(2):
                nc.tensor.matmul(
                    ps, lhs_src[:, 2 * p:2 * p + 2, j0:j0 + 128],
                    rhs_loc[:, 2 * p:2 * p + 2, :],
                    start=(p == 0), stop=(p == 1),
                    perf_mode=DR, skip_group_check=True)

        NWP = wtiles // 2     # window tile pairs

        # ---- Phase C: window pass (same-label masking + positive logits).
        # E tiles are produced now; their Z-matmuls are deferred to the end
        # of the psZ accumulation chain so the global sweep never stalls on
        # the window pass.
        winE = []
        for t, lhsrc in ((0, foTq), (1, fnTq)):
            for wp in range(NWP):
                ps = psS.tile([128, 2 * BL], F32, tag="ps")
                for h in range(2):
                    s = 2 * wp + h
                    mm_group(ps[:, h * BL:(h + 1) * BL], lhsrc, s * 128)
                if t == 0:
                    for s in range(max(2 * wp, LT),
                                   min(2 * wp + 2, LT + 4)):
                        k = s - LT
                        off = (s - 2 * wp) * BL + k * 128
                        tmp = work.tile([128, 128], F32, tag="diag")
                        nc.vector.tensor_mul(out=tmp,
                                             in0=ps[:, off:off + 128],
                                             in1=identS)
                        nc.tensor.matmul(psP[0:1, k * 128:(k + 1) * 128],
                                         ones_f, tmp, start=True, stop=True,
                                         skip_group_check=True)
                for h in range(2):
                    s = 2 * wp + h
                    eqm = work.tile([128, BL], F32, tag="eqm")
                    nc.vector.tensor_scalar(
                        out=eqm, in0=tlb, scalar1=twc[:, s:s + 1],
                        scalar2=NEG, op0=ALU.is_equal, op1=ALU.mult)
                    nc.vector.tensor_add(out=ps[:, h * BL:(h + 1) * BL],
                                         in0=ps[:, h * BL:(h + 1) * BL],
                                         in1=eqm)
                E = wE.tile([128, 2 * BL], BF16, tag="wE")
                nc.scalar.activation(out=E, in_=ps, func=AF.Exp,
                                     bias=ebias, scale=SCALE_Q)
                winE.append(E)

        # ---- Phase D: global sweep; n2o (gTo) first so it only waits on the
        # first AllGather, n2n (gTn) second.  This opens the psZ chain. ----
        first_z = True
        for t, ccout, gT in ((0, ccout_o, gTo), (1, ccout_f, gTn)):
            for r in range(NCORES):
                nc.sync.dma_start(
                    out=gT[:, :, r * BL:(r + 1) * BL],
                    in_=ccout[r].rearrange("(a p) j -> p a j", p=128))
            for gp in range(NGT // 2):
                ps = psS.tile([128, 2 * BL], F32, tag="ps")
                for h in range(2):
                    mm_group(ps[:, h * BL:(h + 1) * BL], gT,
                             (2 * gp + h) * 128)
                E = epool.tile([128, 2 * BL], BF16, tag="E")
                nc.scalar.activation(out=E, in_=ps, func=AF.Exp,
                                     bias=ebias, scale=SCALE_Q)
                for h in range(2):
                    g = 2 * gp + h
                    nc.tensor.matmul(psZ[0:1, :], wcol[:, g:g + 1],
                                     E[:, h * BL:(h + 1) * BL],
                                     start=first_z, stop=False,
                                     skip_group_check=True)
                    first_z = False

        # deferred window Z-matmuls close the psZ chain
        nE = len(winE)
        for i, E in enumerate(winE):
            for h in range(2):
                nc.tensor.matmul(psZ[0:1, :], ones_bf,
                                 E[:, h * BL:(h + 1) * BL],
                                 start=False,
                                 stop=(i == nE - 1 and h == 1),
                                 skip_group_check=True)

        # ---- Phase E: loss tail ----
        lnz = singles.tile([1, BL], F32, tag="lnz")
        nc.scalar.activation(out=lnz, in_=psZ[0:1, :], func=AF.Ln,
                             scale=float(math.exp(-EBIAS)))
        pos100 = singles.tile([1, BL], F32, tag="pos100")
        nc.scalar.activation(out=pos100, in_=psP[0:1, :], func=AF.Copy,
                             scale=SCALE_Q)
        lv = singles.tile([1, BL], F32, tag="lv")
        nc.vector.tensor_sub(out=lv, in0=lnz, in1=pos100)
        part = singles.tile([1, 1], F32, tag="part")
        nc.vector.reduce_sum(out=part, in_=lv, axis=mybir.AxisListType.X)
        nc.sync.dma_start(out=outp[0:1, 0:1], in_=part)

    nc.compile()
    return nc


def make_in_maps(feat, feat_old, targets):
    """Sort rows by label, build the per-core input dicts. Returns
    (in_maps, wtiles)."""
    feat = np.asarray(feat, dtype=np.float32)
    feat_old = np.asarray(feat_old, dtype=np.float32)
    targets_np = np.asarray(targets)

    order = np.argsort(targets_np, kind="stable")
    fs = feat[order].astype(NP_BF16)
    fo = feat_old[order].astype(NP_BF16)
    ts = targets_np[order].astype(np.float32)

    # window padding must cover the largest same-label group
    _, counts = np.unique(targets_np, return_counts=True)
    maxc = int(counts.max()) if counts.size else 1
    lpad_tiles = max(1, -(-(maxc - 1) // 128))
    wtiles = 4 + 2 * lpad_tiles
    LPAD = lpad_tiles * 128
    WIN = wtiles * 128

    idm = np.eye(128, dtype=np.float32)
    in_maps = []
    for c in range(NCORES):
        idx = (np.arange(c * BL - LPAD, c * BL - LPAD + WIN)) % B
        wvec = np.ones(B, dtype=NP_BF16)
        wvec[idx] = 0
        in_maps.append({
            "xw": np.ascontiguousarray(fs[idx]),
            "yw": np.ascontiguousarray(fo[idx]),
            "tw": np.ascontiguousarray(ts[idx]),
            "tl": np.ascontiguousarray(ts[c * BL:(c + 1) * BL]),
            "wv": wvec,
            "idm": idm,
        })
    return in_maps, wtiles


def kernel(feat: np.ndarray, feat_old: np.ndarray,
           targets: np.ndarray) -> np.ndarray:
    in_maps, wtiles = make_in_maps(feat, feat_old, targets)
    if wtiles not in _cache:
        _cache[wtiles] = _build(wtiles)
    nc = _cache[wtiles]

    res = run_bass_kernel_spmd(nc, in_maps, core_ids=list(range(NCORES)))
    total = sum(float(res.results[c]["outp"][0, 0]) for c in range(NCORES))
    return np.asarray(np.float32(total / B))


if __name__ == "__main__":
    rng = np.random.default_rng(0)
    f = rng.standard_normal((B, D)).astype(np.float32)
    g = rng.standard_normal((B, D)).astype(np.float32)
    t = rng.integers(0, 1000, size=B).astype(np.int64)
    print("loss:", kernel(f, g, t))


# revision 13
# speedup vs baseline: 1.0810x; 1.0810x over previous
"""Trainium2 Bass kernel for nn_BackwardCompatibleLoss.

Strategy (data-parallel over batch rows, 8 NeuronCores):

Host side (data movement only):
  - Rows are sorted by target label (the loss is permutation-invariant over
    batch rows).  After sorting, every same-label group is a contiguous row
    range, so for each core's 512-row shard all same-label partners lie in a
    fixed-size "window" of rows around the shard.
  - Each core receives its window of feat/feat_old rows (bf16), the window/
    local targets (f32), a per-core 0/1 weight vector (0 on its window rows)
    and an identity matrix constant.

Device side (all O(B*D) and O(B^2) math):
  - Each core L2-normalizes its window rows (bn_stats -> bn_aggr ->
    64/||x|| = exp(-0.5*ln(ss) + ln 64)).  The exp/ln rsqrt keeps ScalarE on
    a single activation table for the whole kernel (exp/ln/copy share one
    table; sqrt does not, and every table switch costs 1.3us).
    (tensor_tensor_reduce would fuse the sum of squares, but that
    instruction reproducibly hangs the device in this program, so bn_stats
    it is.)
  - Normalized rows are written bf16, DMA-transposed to [D, rows] layout,
    then cast to fp8 e4m3 (values <= 16, inside e4m3 range).
  - The local 512-row slice of feat_old is processed FIRST and AllGathered
    immediately; feat follows in a second AllGather.  The n2o sweep only
    needs the first gather, so it overlaps the second gather; the window
    pass (local data) overlaps the first.
  - Main compute per pair of 128-row j-tiles (transposed orientation
    S^T[j, i]): PSUM q = 4096*S via fp8 DoubleRow matmuls (2 contraction
    slabs per matmul); one exp over the [128, 1024] pair on ScalarE
    (E = exp(q/40.96 - 35); the -35 shift keeps every exponent in range);
    Z[1, 512] accumulates in PSUM via weights-vector matmuls
    (partition-axis reduction on the TensorEngine).
  - Window j-tiles take the same-label additive mask (-1e9), built on-device
    from target equality, before the exp; global-sweep tiles are weighted by
    w (0 on window rows) so each j contributes exactly once.  The window
    pass E tiles are computed early (they only need local data) but their
    Z-matmuls are deferred to the end of the PSUM accumulation chain, so the
    global sweep never waits on them.
  - The positive logit is the diagonal of the window n2o product (identity
    mask + ones-matmul).  loss_i = ln(Z_i) + 35 - q_pos_i/40.96, summed to a
    per-core partial.

  Top-k(1024) in the reference is replaced by the full masked logsumexp: with
  temperature 0.01 the excluded tail contributes ~2e-6 relative error.  The
  fp8 feature quantization adds ~1e-3 relative error on the loss.

Host sums the 8 partial outputs -> mean.
"""

import sys

if "/opt/trn_rl_repo" not in sys.path:
    sys.path.insert(0, "/opt/trn_rl_repo")

import math
from contextlib import ExitStack

import numpy as np

import concourse.bacc as bacc
import concourse.bass as bass
import concourse.tile as tile
from concourse import mybir
from concourse.bass_utils import run_bass_kernel_spmd

F32 = mybir.dt.float32
BF16 = mybir.dt.bfloat16
FP8 = mybir.dt.float8e4
NP_BF16 = mybir.dt.np(BF16)
AF = mybir.ActivationFunctionType
ALU = mybir.AluOpType
DR = mybir.MatmulPerfMode.DoubleRow

B, D = 4096, 512
NCORES = 8
BL = B // NCORES          # 512 local rows per core
NDB = D // 128            # 4 contraction blocks
NGT = B // 128            # 32 global j-tiles
TEMP = 0.01
QS = 64.0                 # fp8 feature scale: q = 4096 * S
SCALE_Q = (1.0 / TEMP) / (QS * QS)   # exp scale on raw psum
EBIAS = -35.0             # exp(q*SCALE_Q - 35): keeps exponents in f32 range
NEG = -1.0e9

_cache = {}


def _build(wtiles: int):
    """Build + compile the SPMD program. wtiles = window size in 128-row tiles."""
    WIN = wtiles * 128
    LPAD = ((wtiles - 4) // 2) * 128          # rows of left padding in window
    LT = LPAD // 128

    nc = bacc.Bacc("TRN2", target_bir_lowering=False, debug=False,
                   num_devices=NCORES)

    xw = nc.dram_tensor("xw", [WIN, D], BF16, kind="ExternalInput")
    yw = nc.dram_tensor("yw", [WIN, D], BF16, kind="ExternalInput")
    tw = nc.dram_tensor("tw", [WIN], F32, kind="ExternalInput")
    tl = nc.dram_tensor("tl", [BL], F32, kind="ExternalInput")
    wv = nc.dram_tensor("wv", [B], BF16, kind="ExternalInput")
    idm = nc.dram_tensor("idm", [128, 128], F32, kind="ExternalInput")
    outp = nc.dram_tensor("outp", [1, 1], F32, kind="ExternalOutput")

    natf = nc.dram_tensor("natf", [WIN, D], BF16)
    nato = nc.dram_tensor("nato", [WIN, D], BF16)
    ccin = nc.dram_tensor("ccin", [2, D, BL], FP8)
    ccout = nc.dram_tensor("ccout", [NCORES, 2, D, BL], FP8,
                           addr_space="Shared")

    with ExitStack() as ctx:
        tc = ctx.enter_context(tile.TileContext(nc))
        singles = ctx.enter_context(tc.tile_pool(name="singles", bufs=1))
        work = ctx.enter_context(tc.tile_pool(name="work", bufs=3))
        xpool = ctx.enter_context(tc.tile_pool(name="xpool", bufs=5))
        epool = ctx.enter_context(tc.tile_pool(name="epool", bufs=3))
        wE = ctx.enter_context(tc.tile_pool(name="wE", bufs=6))
        psS = ctx.enter_context(tc.tile_pool(name="psS", bufs=3, space="PSUM"))
        psA = ctx.enter_context(tc.tile_pool(name="psA", bufs=1, space="PSUM"))

        # persistent SBUF tensors
        foT = singles.tile([128, NDB, WIN], BF16, tag="foT")
        fnT = singles.tile([128, NDB, WIN], BF16, tag="fnT")
        foTq = singles.tile([128, NDB, WIN], FP8, tag="foTq")
        fnTq = singles.tile([128, NDB, WIN], FP8, tag="fnTq")
        gTo = singles.tile([128, NDB, B], FP8, tag="gTo")
        gTn = singles.tile([128, NDB, B], FP8, tag="gTn")
        tlb = singles.tile([128, BL], F32, tag="tlb")
        twc = singles.tile([128, wtiles], F32, tag="twc")
        wcol = singles.tile([128, NGT], BF16, tag="wcol")
        identS = singles.tile([128, 128], F32, tag="identS")
        ones_bf = singles.tile([128, 1], BF16, tag="ones_bf")
        ones_f = singles.tile([128, 1], F32, tag="ones_f")
        ebias = singles.tile([128, 1], F32, tag="ebias")
        ln64 = singles.tile([128, 1], F32, tag="ln64")
        psZ = psA.tile([1, BL], F32, tag="psZ")
        psP = psA.tile([1, BL], F32, tag="psP")

        nc.vector.memset(ones_bf, 1.0)
        nc.vector.memset(ebias, EBIAS)
        nc.vector.memset(ones_f, 1.0)
        nc.vector.memset(ln64, float(math.log(QS)))
        nc.sync.dma_start(out=identS, in_=idm[:, :])
        tl_ap = tl.ap()
        nc.sync.dma_start(
            out=tlb,
            in_=bass.AP(tensor=tl_ap.tensor, offset=tl_ap.offset,
                        ap=[[0, 128]] + list(tl_ap.ap)),
        )
        nc.sync.dma_start(out=twc, in_=tw.ap().rearrange("(s p) -> p s", p=128))
        nc.sync.dma_start(out=wcol, in_=wv.ap().rearrange("(g p) -> p g", p=128))

        MAGIC = 0x5F3759DF
        RSC = QS / math.sqrt(float(D))

        def norm_group(src, nat, blocks):
            """L2-normalize the given 128-row blocks of src, scaled by QS.
            The rsqrt runs entirely on DVE (quake initial guess + 2 Newton
            steps) so ScalarE only ever runs Exp (one activation table, no
            1.3us table reloads)."""
            nblk = len(blocks)
            exg = work.tile([128, nblk], F32, tag="exg")
            xbs = []
            for k, b in enumerate(blocks):
                xb = xpool.tile([128, D], BF16, tag="xb")
                nc.sync.dma_start(out=xb, in_=src[b * 128:(b + 1) * 128, :])
                st = work.tile([128, 6], F32, tag="st")
                nc.vector.bn_stats(out=st, in_=xb)
                mv = work.tile([128, 2], F32, tag="mv")
                nc.vector.bn_aggr(out=mv, in_=st)
                m2 = work.tile([128, 1], F32, tag="m2")
                nc.vector.tensor_mul(out=m2, in0=mv[:, 0:1], in1=mv[:, 0:1])
                nc.vector.tensor_add(out=exg[:, k:k + 1], in0=m2,
                                     in1=mv[:, 1:2])
                xbs.append(xb)
            # u = exg = ||x||^2 / D;  y ~= rsqrt(u) via quake + 2 Newton
            yq = work.tile([128, nblk], F32, tag="yq")
            nc.vector.tensor_scalar(
                out=yq.bitcast(mybir.dt.uint32), in0=exg.bitcast(mybir.dt.uint32),
                scalar1=1, scalar2=None, op0=ALU.logical_shift_right)
            nc.vector.tensor_scalar(
                out=yq.bitcast(mybir.dt.uint32), in0=yq.bitcast(mybir.dt.uint32),
                scalar1=0xFFFFFFFF, scalar2=None, op0=ALU.bitwise_xor)
            # signed add: the uint32 ALU add saturates instead of wrapping
            nc.vector.tensor_scalar(
                out=yq.bitcast(mybir.dt.int32), in0=yq.bitcast(mybir.dt.int32),
                scalar1=MAGIC + 1, scalar2=None, op0=ALU.add)
            rsg = work.tile([128, nblk], F32, tag="rsg")
            cur = yq
            for it in range(2):
                y2 = work.tile([128, nblk], F32, tag="y2")
                nc.vector.tensor_mul(out=y2, in0=cur, in1=cur)
                nc.vector.tensor_mul(out=y2, in0=y2, in1=exg)
                nc.vector.tensor_scalar(
                    out=y2, in0=y2, scalar1=-0.5, scalar2=1.5,
                    op0=ALU.mult, op1=ALU.add)
                dst = rsg if it == 1 else yq
                nc.vector.tensor_mul(out=dst, in0=cur, in1=y2)
                cur = dst
            # rs = RSC * rsqrt(u) = QS / ||x||
            nc.vector.tensor_scalar_mul(out=rsg, in0=rsg, scalar1=RSC)
            for k, b in enumerate(blocks):
                nb = xpool.tile([128, D], BF16, tag="nb")
                nc.vector.tensor_scalar_mul(out=nb, in0=xbs[k],
                                            scalar1=rsg[:, k:k + 1])
                nc.sync.dma_start(out=nat[b * 128:(b + 1) * 128, :], in_=nb)
        def transpose_rows(nat, dstT, r0, r1):
            for db in range(NDB):
                nc.sync.dma_start_transpose(
                    out=dstT[:, db, r0:r1],
                    in_=nat[r0:r1, db * 128:(db + 1) * 128])

        def cast_q(srcT, dstTq, r0, r1):
            nc.scalar.activation(out=dstTq[:, :, r0:r1],
                                 in_=srcT[:, :, r0:r1], func=AF.Copy)

        # ---- Phase A+B: local rows of both tensors, one AllGather of the
        # combined fp8 block; pads afterwards ----
        for t, (src, nat, natT, natTq) in enumerate((
                (yw, nato, foT, foTq),
                (xw, natf, fnT, fnTq))):
            norm_group(src, nat, list(range(LT, LT + 4)))
            transpose_rows(nat, natT, LPAD, LPAD + BL)
            cast_q(natT, natTq, LPAD, LPAD + BL)
            nc.sync.dma_start(
                out=ccin[t].rearrange("(a p) j -> p a j", p=128),
                in_=natTq[:, :, LPAD:LPAD + BL])
        nc.gpsimd.collective_compute(
            "AllGather",
            ALU.bypass,
            replica_groups=[list(range(NCORES))],
            ins=[ccin.ap().opt()],
            outs=[ccout.ap().opt()],
        )
        pads = [s for s in range(wtiles) if not (LT <= s < LT + 4)]
        for src, nat, natT, natTq in ((yw, nato, foT, foTq),
                                      (xw, natf, fnT, fnTq)):
            norm_group(src, nat, pads)
            for s in pads:
                transpose_rows(nat, natT, s * 128, (s + 1) * 128)
                cast_q(natT, natTq, s * 128, (s + 1) * 128)

        rhs_loc = fnTq[:, :, LPAD:LPAD + BL]   # [128, NDB, 512] local fn cols

        def mm_group(ps, lhs_src, j0):
            """ps[j 128, i 512] = sum_d lhs_src[d, j0:j0+128] * local fn."""
            for p in range(2):
                nc.tensor.matmul(
                    ps, lhs_src[:, 2 * p:2 * p + 2, j0:j0 + 128],
                    rhs_loc[:, 2 * p:2 * p + 2, :],
                    start=(p == 0), stop=(p == 1),
                    perf_mode=DR, skip_group_check=True)

        NWP = wtiles // 2     # window tile pairs

        # ---- Phase C: window pass (same-label masking + positive logits).
        # E tiles are produced now; their Z-matmuls are deferred to the end
        # of the psZ accumulation chain so the global sweep never stalls on
        # the window pass.
        winE = []
        for t, lhsrc in ((0, foTq), (1, fnTq)):
            for wp in range(NWP):
                ps = psS.tile([128, 2 * BL], F32, tag="ps")
                for h in range(2):
                    s = 2 * wp + h
                    mm_group(ps[:, h * BL:(h + 1) * BL], lhsrc, s * 128)
                if t == 0:
                    for s in range(max(2 * wp, LT),
                                   min(2 * wp + 2, LT + 4)):
                        k = s - LT
                        off = (s - 2 * wp) * BL + k * 128
                        tmp = work.tile([128, 128], F32, tag="diag")
                        nc.vector.tensor_mul(out=tmp,
                                             in0=ps[:, off:off + 128],
                                             in1=identS)
                        nc.tensor.matmul(psP[0:1, k * 128:(k + 1) * 128],
                                         ones_f, tmp, start=True, stop=True,
                                         skip_group_check=True)
                for h in range(2):
                    s = 2 * wp + h
                    eqm = work.tile([128, BL], F32, tag="eqm")
                    nc.vector.tensor_scalar(
                        out=eqm, in0=tlb, scalar1=twc[:, s:s + 1],
                        scalar2=NEG, op0=ALU.is_equal, op1=ALU.mult)
                    nc.vector.tensor_add(out=ps[:, h * BL:(h + 1) * BL],
                                         in0=ps[:, h * BL:(h + 1) * BL],
                                         in1=eqm)
                E = wE.tile([128, 2 * BL], BF16, tag="wE")
                nc.scalar.activation(out=E, in_=ps, func=AF.Exp,
                                     bias=ebias, scale=SCALE_Q)
                winE.append(E)

        # ---- Phase D: global sweep; n2o (gTo) first so it only waits on the
        # first AllGather, n2n (gTn) second.  This opens the psZ chain. ----
        first_z = True
        for t, gT in ((0, gTo), (1, gTn)):
            for r in range(NCORES):
                nc.sync.dma_start(
                    out=gT[:, :, r * BL:(r + 1) * BL],
                    in_=ccout[r, t].rearrange("(a p) j -> p a j", p=128))
            for gp in range(NGT // 2):
                ps = psS.tile([128, 2 * BL], F32, tag="ps")
                for h in range(2):
                    mm_group(ps[:, h * BL:(h + 1) * BL], gT,
                             (2 * gp + h) * 128)
                E = epool.tile([128, 2 * BL], BF16, tag="E")
                nc.scalar.activation(out=E, in_=ps, func=AF.Exp,
                                     bias=ebias, scale=SCALE_Q)
                for h in range(2):
                    g = 2 * gp + h
                    nc.tensor.matmul(psZ[0:1, :], wcol[:, g:g + 1],
                                     E[:, h * BL:(h + 1) * BL],
                                     start=first_z, stop=False,
                                     skip_group_check=True)
                    first_z = False

        # deferred window Z-matmuls close the psZ chain
        nE = len(winE)
        for i, E in enumerate(winE):
            for h in range(2):
                nc.tensor.matmul(psZ[0:1, :], ones_bf,
                                 E[:, h * BL:(h + 1) * BL],
                                 start=False,
                                 stop=(i == nE - 1 and h == 1),
                                 skip_group_check=True)

        # ---- Phase E: loss tail ----
        lnz = singles.tile([1, BL], F32, tag="lnz")
        nc.scalar.activation(out=lnz, in_=psZ[0:1, :], func=AF.Ln,
                             scale=float(math.exp(-EBIAS)))
        pos100 = singles.tile([1, BL], F32, tag="pos100")
        nc.scalar.activation(out=pos100, in_=psP[0:1, :], func=AF.Copy,
                             scale=SCALE_Q)
        lv = singles.tile([1, BL], F32, tag="lv")
        nc.vector.tensor_sub(out=lv, in0=lnz, in1=pos100)
        part = singles.tile([1, 1], F32, tag="part")
        nc.vector.reduce_sum(out=part, in_=lv, axis=mybir.AxisListType.X)
        nc.sync.dma_start(out=outp[0:1, 0:1], in_=part)

    nc.compile()
    return nc


def make_in_maps(feat, feat_old, targets):
    """Sort rows by label, build the per-core input dicts. Returns
    (in_maps, wtiles)."""
    feat = np.asarray(feat, dtype=np.float32)
    feat_old = np.asarray(feat_old, dtype=np.float32)
    targets_np = np.asarray(targets)

    order = np.argsort(targets_np, kind="stable")
    fs = feat[order].astype(NP_BF16)
    fo = feat_old[order].astype(NP_BF16)
    ts = targets_np[order].astype(np.float32)

    # window padding must cover the largest same-label group
    _, counts = np.unique(targets_np, return_counts=True)
    maxc = int(counts.max()) if counts.size else 1
    lpad_tiles = max(1, -(-(maxc - 1) // 128))
    wtiles = 4 + 2 * lpad_tiles
    LPAD = lpad_tiles * 128
    WIN = wtiles * 128

    idm = np.eye(128, dtype=np.float32)
    in_maps = []
    for c in range(NCORES):
        idx = (np.arange(c * BL - LPAD, c * BL - LPAD + WIN)) % B
        wvec = np.ones(B, dtype=NP_BF16)
        wvec[idx] = 0
        in_maps.append({
            "xw": np.ascontiguousarray(fs[idx]),
            "yw": np.ascontiguousarray(fo[idx]),
            "tw": np.ascontiguousarray(ts[idx]),
            "tl": np.ascontiguousarray(ts[c * BL:(c + 1) * BL]),
            "wv": wvec,
            "idm": idm,
        })
    return in_maps, wtiles


def kernel(feat: np.ndarray, feat_old: np.ndarray,
           targets: np.ndarray) -> np.ndarray:
    in_maps, wtiles = make_in_maps(feat, feat_old, targets)
    if wtiles not in _cache:
        _cache[wtiles] = _build(wtiles)
    nc = _cache[wtiles]

    res = run_bass_kernel_spmd(nc, in_maps, core_ids=list(range(NCORES)))
    total = sum(float(res.results[c]["outp"][0, 0]) for c in range(NCORES))
    return np.asarray(np.float32(total / B))


if __name__ == "__main__":
    rng = np.random.default_rng(0)
    f = rng.standard_normal((B, D)).astype(np.float32)
    g = rng.standard_normal((B, D)).astype(np.float32)
    t = rng.integers(0, 1000, size=B).astype(np.int64)
    print("loss:", kernel(f, g, t))
